# revision 1
# baseline (speedup 1.0000x reference)
"""Trainium2 Bass kernel for nn_MessageAggregator (gnn_message_passing).

Computation (reference):
    s   = logsig(logsig(state @ W1_m.T + b1_m) @ W2_m.T)      # [E, D]
    agg = mask_transpose @ (mask @ s) - s                     # [E, D]
    out = logsig(logsig([agg, feature] @ W1_a.T + b1_a) @ W2_a.T)

Sharding: edge dimension E=32768 split across 8 cores (4096 edges each).
Each core:
  phase 0: memory-MLP on its edge slice (feature-major via PE transposes)
  phase 1: partial per-node aggregate  v = -(s.T @ mT_slice)  [D, N]
  AllReduce(v) over the 8 cores
  phase 2: edge aggregate  -(v.T)@mask_slice, subtract -s.T, concat-MLP,
           transpose-free edge-major final matmul, DMA out.

All matmuls run as float32r (fp32 bits, round-robin PE feed, full rate at
moving free dim >= 256).  log_sigmoid(x) = -softplus(-x) is computed
overflow-safely as softplus(t) = max(t,0) + ln(1 + exp(-|t|)) using the
Exp+Ln ACT table (z-values here reach +-5000, so exp(t) would overflow).
Sign bookkeeping keeps intermediates negated (u = -h) so each activation
is a single softplus; weight matrices are transposed/negated on device.
"""

import ml_dtypes
import numpy as np

N_CORES = 8
E, N, D, DF = 32768, 2048, 128, 32
EL = E // N_CORES          # 4096 edges per core
NT = EL // 128             # 32 edge tiles of 128
NCH = EL // 512            # 8 chunks of 512 edges
P = 128

_CACHE: dict = {}


def _build():
    from concourse import bacc, mybir, tile

    F32 = mybir.dt.float32
    F32R = mybir.dt.float32r
    AF = mybir.ActivationFunctionType
    ALU = mybir.AluOpType

    nc = bacc.Bacc("TRN2", target_bir_lowering=False, debug=False,
                   num_devices=N_CORES)

    stateT_l = nc.dram_tensor("stateT_l", [D, EL], mybir.dt.bfloat16, kind="ExternalInput")
    featT_l = nc.dram_tensor("featT_l", [DF, EL], mybir.dt.bfloat16, kind="ExternalInput")
    mT_l = nc.dram_tensor("mT_l", [EL, N], F32, kind="ExternalInput")
    mask_l = nc.dram_tensor("mask_l", [N, EL], F32, kind="ExternalInput")
    w1m = nc.dram_tensor("w1m", [D, D], F32, kind="ExternalInput")
    b1m = nc.dram_tensor("b1m", [D], F32, kind="ExternalInput")
    w2m = nc.dram_tensor("w2m", [D, D], F32, kind="ExternalInput")
    w1a = nc.dram_tensor("w1a", [D, D + DF], F32, kind="ExternalInput")
    b1a = nc.dram_tensor("b1a", [D], F32, kind="ExternalInput")
    w2a = nc.dram_tensor("w2a", [D, D], F32, kind="ExternalInput")
    idn = nc.dram_tensor("idn", [P, P], F32, kind="ExternalInput")
    out_l = nc.dram_tensor("out_l", [EL, D], F32, kind="ExternalOutput")

    with tile.TileContext(nc) as tc:
        with (
            tc.tile_pool(name="consts", bufs=1) as consts,
            tc.tile_pool(name="persist", bufs=1) as persist,
            tc.tile_pool(name="tmp", bufs=2) as tmp,
            tc.tile_pool(name="streamp", bufs=20) as streamp,
            tc.tile_pool(name="outp", bufs=2) as outp,
            tc.tile_pool(name="ps_acc", bufs=1, space="PSUM") as ps_acc,
            tc.tile_pool(name="ps_mm", bufs=2, space="PSUM") as ps_mm,
            tc.tile_pool(name="ps_tp", bufs=2, space="PSUM") as ps_tp,
            tc.tile_pool(name="dram", bufs=1, space="DRAM") as dram,
        ):
            # ---------------- constants & weight prep ----------------
            idn_sb = consts.tile([P, P], F32)
            nc.sync.dma_start(idn_sb[:], idn[:])
            w1m_raw = consts.tile([D, D], F32)
            nc.sync.dma_start(w1m_raw[:], w1m[:])
            w2m_raw = consts.tile([D, D], F32)
            nc.sync.dma_start(w2m_raw[:], w2m[:])
            w1a_raw = consts.tile([D, D + DF], F32)
            nc.sync.dma_start(w1a_raw[:], w1a[:])
            w2a_raw = consts.tile([D, D], F32)
            nc.sync.dma_start(w2a_raw[:], w2a[:])
            b1m_sb = consts.tile([D, 1], F32)
            nc.sync.dma_start(b1m_sb[:], b1m[:, None])
            b1a_sb = consts.tile([D, 1], F32)
            nc.sync.dma_start(b1a_sb[:], b1a[:, None])

            tpw = ps_tp.tile([P, 512], F32, tag="tp")
            nc.tensor.transpose(tpw[:, 0:128], w1m_raw[:], idn_sb[:])
            nc.tensor.transpose(tpw[:, 128:256], w2m_raw[:], idn_sb[:])
            nc.tensor.transpose(tpw[:, 256:384], w1a_raw[:, 0:D], idn_sb[:])
            nc.tensor.transpose(tpw[:, 384:512], w2a_raw[:], idn_sb[:])
            w1mT = consts.tile([D, D], mybir.dt.bfloat16)       # W1m.T
            nc.vector.tensor_copy(w1mT[:], tpw[:, 0:128])
            w2mnT = consts.tile([D, D], F32R)      # -(W2m.T)
            nc.vector.tensor_scalar_mul(w2mnT[:], tpw[:, 128:256], -1.0)
            w1anT = consts.tile([D, D], F32R)      # -(W1a[:, :D].T)
            nc.vector.tensor_scalar_mul(w1anT[:], tpw[:, 256:384], -1.0)
            w2anT = consts.tile([D, D], F32R)      # -(W2a.T)
            nc.vector.tensor_scalar_mul(w2anT[:], tpw[:, 384:512], -1.0)
            tpw2 = ps_tp.tile([P, 512], F32, tag="tp")
            nc.tensor.transpose(tpw2[:DF, 0:128], w1a_raw[:, D:], idn_sb[:])
            wa2T = consts.tile([DF, D], mybir.dt.bfloat16)  # W1a[:, D:].T
            nc.vector.tensor_copy(wa2T[:], tpw2[:DF, 0:128])
            idn_bf = consts.tile([P, P], mybir.dt.bfloat16)
            nc.vector.tensor_copy(idn_bf[:], idn_sb[:])

            # ---------------- persistent intermediates ----------------
            u2T = persist.tile([P, EL], mybir.dt.bfloat16)  # -s.T (feat-major)
            u2e = persist.tile([P, NT, D], F32R)       # -s    (edge-major tiles)
            featT = persist.tile([DF, EL], mybir.dt.bfloat16)  # feature.T
            vT = persist.tile([P, N // P, D], F32R)    # -agg   [n, da] tiles

            stateT_sb = persist.tile([P, EL], mybir.dt.bfloat16)
            for q4 in range(4):
                nc.sync.dma_start(
                    stateT_sb[:, q4 * 1024 : (q4 + 1) * 1024],
                    stateT_l[:, q4 * 1024 : (q4 + 1) * 1024],
                )
            nc.sync.dma_start(featT[:], featT_l[:])

            def softplus(z_ps, bias_ap, out_ap, w=512):
                """out = softplus(-z_ps - bias): 3 DVE + 2 ACT, overflow-safe."""
                t = tmp.tile([P, w], F32, tag="t")
                a = tmp.tile([P, w], F32, tag="a")
                if bias_ap is not None:
                    nc.vector.tensor_scalar(
                        t[:], z_ps, -1.0, bias_ap, ALU.mult, ALU.subtract
                    )
                else:
                    nc.vector.tensor_scalar_mul(t[:], z_ps, -1.0)
                nc.vector.tensor_scalar(
                    a[:].bitcast(mybir.dt.uint32),
                    t[:].bitcast(mybir.dt.uint32),
                    0x7FFFFFFF, None, ALU.bitwise_and,
                )
                ex = tmp.tile([P, w], F32, tag="ex")
                nc.scalar.activation(ex[:], a[:], AF.Exp, scale=-1.0)
                ln = tmp.tile([P, w], F32, tag="ln")
                nc.scalar.activation(ln[:], ex[:], AF.Ln, bias=1.0)
                nc.vector.scalar_tensor_tensor(
                    out_ap, t[:], 0.0, ln[:], ALU.max, ALU.add
                )

            # negated bias for the direct 2-ACT softplus in phase 0
            nb1m_sb = consts.tile([D, 1], F32)
            nc.vector.tensor_scalar_mul(nb1m_sb[:], b1m_sb[:], -1.0)

            # ------- phase 0 (memory MLP) interleaved with phase 1 -------
            # |z| <= ~4 in the memory MLP, so softplus(-z) = Ln(Exp(-z)+1)
            # directly (no overflow guard needed).  Phase-1 accumulators:
            # acc0/acc1 = node cols 0:1024, acc2/acc3 = 1024:2048.
            accs = [
                ps_acc.tile([P, 512], F32, tag=f"acc{q}", name=f"p1acc{q}")
                for q in range(4)
            ]
            for g in range(NCH // 2):
                pj = (2 * g, 2 * g + 1)
                h1s = {}
                for j in pj:
                    h1 = ps_mm.tile([P, 512], F32, tag="mm", name=f"h1_{j}")
                    nc.tensor.matmul(
                        h1[:], w1mT[:], stateT_sb[:, j * 512 : (j + 1) * 512],
                        start=True, stop=True,
                    )
                    h1s[j] = h1
                ex1s = {}
                for j in pj:
                    ex1 = tmp.tile([P, 512], F32, tag="ex", name=f"ex1_{j}")
                    nc.scalar.activation(ex1[:], h1s[j][:], AF.Exp,
                                         scale=-1.0, bias=nb1m_sb[:])
                    ex1s[j] = ex1
                u1s = {}
                for j in pj:
                    u1 = tmp.tile([P, 512], F32R, tag="u1", name=f"u1_{j}")
                    nc.scalar.activation(u1[:], ex1s[j][:], AF.Ln, bias=1.0)
                    u1s[j] = u1
                z2s = {}
                for j in pj:
                    z2 = ps_mm.tile([P, 512], F32, tag="mm", name=f"z2_{j}")
                    nc.tensor.matmul(z2[:], w2mnT[:], u1s[j][:],
                                     start=True, stop=True)
                    z2s[j] = z2
                ex2s = {}
                for j in pj:
                    ex2 = tmp.tile([P, 512], F32, tag="ln", name=f"ex2_{j}")
                    nc.scalar.activation(ex2[:], z2s[j][:], AF.Exp, scale=-1.0)
                    ex2s[j] = ex2
                for j in pj:
                    nc.scalar.activation(
                        u2T[:, j * 512 : (j + 1) * 512], ex2s[j][:],
                        AF.Ln, bias=1.0,
                    )
                for j in pj:
                    tp2 = ps_tp.tile([P, 512], mybir.dt.bfloat16, tag="tp",
                                     name=f"tp2_{j}")
                    for k in range(4):
                        c0 = (j * 4 + k) * P
                        nc.tensor.transpose(
                            tp2[:, k * P : (k + 1) * P],
                            u2T[:, c0 : c0 + P],
                            idn_bf[:],
                        )
                    nc.vector.tensor_copy(
                        u2e[:, j * 4 : (j + 1) * 4, :].rearrange(
                            "p a d -> p (a d)"
                        ),
                        tp2[:],
                    )
                    for t_i in range(4 * j, 4 * j + 4):
                        for nh in range(2):
                            mt = streamp.tile([P, 1024], F32R, tag="sp",
                                              name=f"mt_{t_i}_{nh}")
                            nc.sync.dma_start(
                                mt[:],
                                mT_l[
                                    t_i * P : (t_i + 1) * P,
                                    nh * 1024 : (nh + 1) * 1024,
                                ].bitcast(F32R),
                            )
                            for q in range(2):
                                nc.tensor.matmul(
                                    accs[2 * nh + q][:],
                                    u2e[:, t_i, :],
                                    mt[:, q * 512 : (q + 1) * 512],
                                    start=(t_i == 0),
                                    stop=(t_i == NT - 1),
                                )

            # ---------------- AllReduce (single, bf16) ----------------
            vsb = persist.tile([P, N], mybir.dt.bfloat16)
            for q in range(4):
                nc.vector.tensor_copy(
                    vsb[:, q * 512 : (q + 1) * 512], accs[q][:]
                )
            cc_in = dram.tile([P, N], mybir.dt.bfloat16)
            cc_out = dram.tile([P, N], mybir.dt.bfloat16, addr_space="Shared")
            for hv in range(4):
                nc.gpsimd.dma_start(
                    cc_in[:, hv * 512 : (hv + 1) * 512],
                    vsb[:, hv * 512 : (hv + 1) * 512],
                )
            nc.gpsimd.collective_compute(
                "AllReduce",
                mybir.AluOpType.add,
                ins=[cc_in.opt()],
                outs=[cc_out.opt()],
                replica_groups=[list(range(N_CORES))],
            )
            vfull = persist.tile([P, N], mybir.dt.bfloat16)
            for hv in range(4):
                nc.gpsimd.dma_start(
                    vfull[:, hv * 512 : (hv + 1) * 512],
                    cc_out[:, hv * 512 : (hv + 1) * 512],
                )

            for g in range(4):
                tp3 = ps_tp.tile([P, 512], mybir.dt.bfloat16, tag="tp",
                                 name=f"tp3_{g}")
                for k in range(4):
                    i = g * 4 + k
                    nc.tensor.transpose(
                        tp3[:, k * P : (k + 1) * P],
                        vfull[:, i * P : (i + 1) * P],
                        idn_bf[:],
                    )
                nc.vector.tensor_copy(
                    vT[:, g * 4 : (g + 1) * 4, :].rearrange("p a d -> p (a d)"),
                    tp3[:],
                )

            # ---------------- phase 2: edge agg + concat MLP ----------------
            out_v = out_l.rearrange("(c k p) d -> c p k d", k=4, p=P)

            def p2_mask_mm(j, acc, nch_range):
                for nch in nch_range:
                    mk = maskp.tile([P, 512], F32R, tag="mk",
                                    name=f"mk_{j}_{nch}")
                    nc.sync.dma_start(
                        mk[:],
                        mask_l[
                            nch * P : (nch + 1) * P, j * 512 : (j + 1) * 512
                        ].bitcast(F32R),
                    )
                    nc.tensor.matmul(
                        acc[:],
                        vT[:, nch, :],
                        mk[:],
                        start=(nch == 0),
                        stop=(nch == N // P - 1),
                    )

            def p2_mlp_pair(jacc):
                w3s, z1as, ts, as_, exs, u3s = {}, {}, {}, {}, {}, {}
                for j, acc in jacc:
                    w3 = tmp.tile([P, 512], F32R, tag="w3", name=f"w3_{j}")
                    nc.vector.tensor_sub(
                        w3[:], acc[:], u2T[:, j * 512 : (j + 1) * 512]
                    )
                    w3s[j] = w3
                for j, acc in jacc:
                    z1a = ps_mm.tile([P, 512], F32, tag="mm", name=f"z1a_{j}")
                    nc.tensor.matmul(z1a[:], w1anT[:], w3s[j][:],
                                     start=True, stop=False)
                    nc.tensor.matmul(
                        z1a[:], wa2T[:], featT[:, j * 512 : (j + 1) * 512],
                        start=False, stop=True,
                    )
                    z1as[j] = z1a
                for j, acc in jacc:
                    t = tmp.tile([P, 512], F32, tag="t", name=f"t_{j}")
                    nc.vector.tensor_scalar(
                        t[:], z1as[j][:], -1.0, b1a_sb[:], ALU.mult,
                        ALU.subtract,
                    )
                    a = tmp.tile([P, 512], F32, tag="a", name=f"a_{j}")
                    nc.vector.tensor_scalar(
                        a[:].bitcast(mybir.dt.uint32),
                        t[:].bitcast(mybir.dt.uint32),
                        0x7FFFFFFF, None, ALU.bitwise_and,
                    )
                    ts[j], as_[j] = t, a
                for j, acc in jacc:
                    ex = tmp.tile([P, 512], F32, tag="ex", name=f"exa_{j}")
                    nc.scalar.activation(ex[:], as_[j][:], AF.Exp, scale=-1.0)
                    exs[j] = ex
                for j, acc in jacc:
                    ln = tmp.tile([P, 512], F32, tag="ln", name=f"lna_{j}")
                    nc.scalar.activation(ln[:], exs[j][:], AF.Ln, bias=1.0)
                    u3 = tmp.tile([P, 512], F32R, tag="u3", name=f"u3_{j}")
                    nc.vector.scalar_tensor_tensor(
                        u3[:], ts[j][:], 0.0, ln[:], ALU.max, ALU.add
                    )
                    u3s[j] = u3
                pos, a2s, e2s = {}, {}, {}
                for j, acc in jacc:
                    po = ps_mm.tile([P, 512], F32, tag="mm", name=f"po_{j}")
                    for k in range(4):
                        nc.tensor.matmul(
                            po[:, k * P : (k + 1) * P],
                            u3s[j][:, k * P : (k + 1) * P],
                            w2anT[:],
                            start=True,
                            stop=True,
                        )
                    pos[j] = po
                    a2 = tmp.tile([P, 512], F32, tag="a", name=f"a2_{j}")
                    nc.vector.tensor_scalar(
                        a2[:].bitcast(mybir.dt.uint32),
                        po[:].bitcast(mybir.dt.uint32),
                        0x7FFFFFFF, None, ALU.bitwise_and,
                    )
                    a2s[j] = a2
                for j, acc in jacc:
                    e2 = tmp.tile([P, 512], F32, tag="ex", name=f"e2_{j}")
                    nc.scalar.activation(e2[:], a2s[j][:], AF.Exp, scale=-1.0)
                    e2s[j] = e2
                for j, acc in jacc:
                    l2 = tmp.tile([P, 512], F32, tag="ln", name=f"l2_{j}")
                    nc.scalar.activation(l2[:], e2s[j][:], AF.Ln, bias=1.0)
                    ob = outp.tile([P, 512], F32, tag="ob", name=f"ob_{j}")
                    nc.vector.scalar_tensor_tensor(
                        ob[:], pos[j][:], 0.0, l2[:], ALU.min, ALU.subtract
                    )
                    nc.gpsimd.dma_start(
                        out_v[j], ob.rearrange("p (k d) -> p k d", k=4)
                    )

            # 2-chunk waves: one [128,1024] tile per n-chunk row
            for w in range(4):
                js = (2 * w, 2 * w + 1)
                acc_w = {
                    j: ps_acc.tile([P, 512], F32, tag=f"acc{j % 4}",
                                   name=f"p2acc_{j}")
                    for j in js
                }
                for nch in range(16):
                    mk = streamp.tile([P, 1024], F32R, tag="sp",
                                      name=f"mk_{w}_{nch}")
                    nc.sync.dma_start(
                        mk[:],
                        mask_l[
                            nch * P : (nch + 1) * P,
                            2 * w * 512 : (2 * w + 2) * 512,
                        ].bitcast(F32R),
                    )
                    for ji, j in enumerate(js):
                        nc.tensor.matmul(
                            acc_w[j][:],
                            vT[:, nch, :],
                            mk[:, ji * 512 : (ji + 1) * 512],
                            start=(nch == 0),
                            stop=(nch == 15),
                        )
                p2_mlp_pair([(j, acc_w[j]) for j in js])
    nc.compile()
    return nc


def kernel(**inputs: np.ndarray) -> np.ndarray:
    from concourse.bass_utils import run_bass_kernel_spmd

    if "nc" not in _CACHE:
        _CACHE["nc"] = _build()
    nc = _CACHE["nc"]

    state = np.ascontiguousarray(inputs["state"], dtype=np.float32)
    feature = np.ascontiguousarray(inputs["feature"], dtype=np.float32)
    mask = np.ascontiguousarray(inputs["mask"], dtype=np.float32)
    mask_transpose = np.ascontiguousarray(
        inputs["mask_transpose"], dtype=np.float32
    )
    idn_np = np.eye(P, dtype=np.float32)

    common = {
        "w1m": np.ascontiguousarray(inputs["W1_m"], dtype=np.float32),
        "b1m": np.ascontiguousarray(inputs["b1_m"], dtype=np.float32),
        "w2m": np.ascontiguousarray(inputs["W2_m"], dtype=np.float32),
        "w1a": np.ascontiguousarray(inputs["W1_a"], dtype=np.float32),
        "b1a": np.ascontiguousarray(inputs["b1_a"], dtype=np.float32),
        "w2a": np.ascontiguousarray(inputs["W2_a"], dtype=np.float32),
        "idn": idn_np,
    }
    in_maps = []
    for c in range(N_CORES):
        sl = slice(c * EL, (c + 1) * EL)
        in_maps.append(
            {
                "stateT_l": np.ascontiguousarray(state[sl].T).astype(
                    ml_dtypes.bfloat16
                ),
                "featT_l": np.ascontiguousarray(feature[sl].T).astype(
                    ml_dtypes.bfloat16
                ),
                "mT_l": mask_transpose[sl],
                "mask_l": np.ascontiguousarray(mask[:, sl]),
                **common,
            }
        )
    _CACHE["in_maps"] = in_maps

    res = run_bass_kernel_spmd(nc, in_maps, core_ids=list(range(N_CORES)))
    out = np.concatenate(
        [res.results[c]["out_l"] for c in range(N_CORES)], axis=0
    )
    return out



# revision 7
# speedup vs baseline: 1.6660x; 1.6660x over previous
"""Trainium2 Bass kernel for nn_MessageAggregator (gnn_message_passing).

Computation (reference):
    s   = logsig(logsig(state @ W1_m.T + b1_m) @ W2_m.T)      # [E, D]
    agg = mask_transpose @ (mask @ s) - s                     # [E, D]
    out = logsig(logsig([agg, feature] @ W1_a.T + b1_a) @ W2_a.T)

Sharding: edge dimension E=32768 split across 8 cores (4096 edges each).

Key optimizations over the fp32 baseline:
  - mask / mask_transpose streamed as fp8 e4m3 (0/1 values are exact)
    -> 4x less HBM traffic on the dominant streams.
  - mixed-dtype matmuls: bf16 stationary (s / v tiles) x fp8 moving
    (mask tiles); PE runs fp8 at bf16 rate, full precision kept in the
    data operand.
  - one ACT table load total: the act-table placement pass greedily
    picks the first set containing each function (Exp -> exp_and_others,
    Ln -> natural_log), reloading tables on every switch (40x in the
    baseline = 51us).  We patch the table list it sees so only
    natural_log_exp_and_others (real index preserved) claims Exp/Ln.
    Negated-intermediate bookkeeping (u = -h) folds all negations into
    pre-negated weight matrices; softplus(t) = max(t,0) + Ln(1+Exp(-|t|))
    at the wide-range phase-2 sites, direct Ln(1+Exp(t)) in phase 0.
  - full mask [N, EL] (8 MB fp8) is prefetched into SBUF behind the
    mask_transpose stream so the AllReduce hides under DMA, and phase 2
    runs out of SBUF.
"""

import ml_dtypes
import numpy as np

N_CORES = 8
E, N, D, DF = 32768, 2048, 128, 32
EL = E // N_CORES          # 4096 edges per core
NT = EL // 128             # 32 edge tiles of 128
NCH = EL // 512            # 8 chunks of 512 edges
NNCH = N // 128            # 16 node chunks of 128
P = 128

_CACHE: dict = {}


def _build():
    from concourse import bacc, mybir, tile
    import concourse.hw_specs as hw_specs

    F32 = mybir.dt.float32
    BF16 = mybir.dt.bfloat16
    FP8 = mybir.dt.float8e4
    AF = mybir.ActivationFunctionType
    ALU = mybir.AluOpType

    # Make the act-table placement pass resolve both Exp and Ln to the
    # combined natural_log_exp_and_others set (its real act_info index is
    # preserved, so the runtime loads the correct TDRAM tables).  Only the
    # placement decision changes: 1 table load instead of ping-ponging.
    _orig_tables = hw_specs.get_activation_tables

    def _patched_tables(arch):
        return {
            name: (funcs if name == "natural_log_exp_and_others"
                   else funcs - {AF.Exp, AF.Ln})
            for name, funcs in _orig_tables(arch).items()
        }

    bacc.get_activation_tables = _patched_tables

    nc = bacc.Bacc("TRN2", target_bir_lowering=False, debug=False,
                   num_devices=N_CORES)

    stateT_l = nc.dram_tensor("stateT_l", [D, EL], BF16, kind="ExternalInput")
    featT_l = nc.dram_tensor("featT_l", [DF, EL], BF16, kind="ExternalInput")
    mT_l = nc.dram_tensor("mT_l", [EL, N], FP8, kind="ExternalInput")
    mask_l = nc.dram_tensor("mask_l", [N, EL], FP8, kind="ExternalInput")
    w1m = nc.dram_tensor("w1m", [D, D], F32, kind="ExternalInput")
    b1m = nc.dram_tensor("b1m", [D], F32, kind="ExternalInput")
    w2m = nc.dram_tensor("w2m", [D, D], F32, kind="ExternalInput")
    w1a = nc.dram_tensor("w1a", [D, D + DF], F32, kind="ExternalInput")
    b1a = nc.dram_tensor("b1a", [D], F32, kind="ExternalInput")
    w2a = nc.dram_tensor("w2a", [D, D], F32, kind="ExternalInput")
    idn = nc.dram_tensor("idn", [P, P], F32, kind="ExternalInput")
    out_l = nc.dram_tensor("out_l", [EL, D], F32, kind="ExternalOutput")

    with tile.TileContext(nc) as tc:
        with (
            tc.tile_pool(name="consts", bufs=1) as consts,
            tc.tile_pool(name="persist", bufs=1) as persist,
            tc.tile_pool(name="maskp", bufs=1) as maskp,
            tc.tile_pool(name="tmp", bufs=3) as tmp,
            tc.tile_pool(name="mtp", bufs=6) as mtp,
            tc.tile_pool(name="outp", bufs=2) as outp,
            tc.tile_pool(name="ps_acc", bufs=1, space="PSUM") as ps_acc,
            tc.tile_pool(name="ps_mm", bufs=2, space="PSUM") as ps_mm,
            tc.tile_pool(name="ps_tp", bufs=2, space="PSUM") as ps_tp,
            tc.tile_pool(name="dram", bufs=1, space="DRAM") as dram,
        ):
            # ---------------- constants & weight prep ----------------
            idn_sb = consts.tile([P, P], F32)
            nc.sync.dma_start(idn_sb[:], idn[:])
            w1m_raw = consts.tile([D, D], F32)
            nc.sync.dma_start(w1m_raw[:], w1m[:])
            w2m_raw = consts.tile([D, D], F32)
            nc.sync.dma_start(w2m_raw[:], w2m[:])
            w1a_raw = consts.tile([D, D + DF], F32)
            nc.sync.dma_start(w1a_raw[:], w1a[:])
            w2a_raw = consts.tile([D, D], F32)
            nc.sync.dma_start(w2a_raw[:], w2a[:])
            b1a_sb = consts.tile([D, 1], F32)
            nc.sync.dma_start(b1a_sb[:], b1a[:, None])
            b1m_sb = consts.tile([D, 1], F32)
            nc.sync.dma_start(b1m_sb[:], b1m[:, None])

            tpw = ps_tp.tile([P, 512], F32, tag="tp")
            nc.tensor.transpose(tpw[:, 0:128], w1m_raw[:], idn_sb[:])
            nc.tensor.transpose(tpw[:, 128:256], w2m_raw[:], idn_sb[:])
            nc.tensor.transpose(tpw[:, 256:384], w1a_raw[:, 0:D], idn_sb[:])
            nc.tensor.transpose(tpw[:, 384:512], w2a_raw[:], idn_sb[:])
            w1mT = consts.tile([D, D], BF16)                # W1m.T
            nc.vector.tensor_copy(w1mT[:], tpw[:, 0:128])
            w2mnT = consts.tile([D, D], BF16)               # -(W2m.T)
            nc.vector.tensor_scalar_mul(w2mnT[:], tpw[:, 128:256], -1.0)
            w1anT = consts.tile([D, D], BF16)               # -(W1a[:, :D].T)
            nc.vector.tensor_scalar_mul(w1anT[:], tpw[:, 256:384], -1.0)
            w2anT = consts.tile([D, D], BF16)               # -(W2a.T)
            nc.vector.tensor_scalar_mul(w2anT[:], tpw[:, 384:512], -1.0)
            tpw2 = ps_tp.tile([P, 512], F32, tag="tp")
            nc.tensor.transpose(tpw2[:DF, 0:128], w1a_raw[:, D:], idn_sb[:])
            wa2T = consts.tile([DF, D], BF16)               # W1a[:, D:].T
            nc.vector.tensor_copy(wa2T[:], tpw2[:DF, 0:128])
            idn_bf = consts.tile([P, P], BF16)
            nc.vector.tensor_copy(idn_bf[:], idn_sb[:])
            # negated bias for phase-0 softplus (u1 = softplus(-z - b))
            nb1m_sb = consts.tile([D, 1], F32)
            nc.vector.tensor_scalar_mul(nb1m_sb[:], b1m_sb[:], -1.0)

            # ---------------- persistent intermediates ----------------
            u2T = persist.tile([P, EL], BF16)       # -s.T (feat-major)
            u2e = persist.tile([P, NT, D], BF16)    # -s   (edge-major tiles)
            featT = persist.tile([DF, EL], BF16)    # feature.T
            vT = persist.tile([P, NNCH, D], BF16)   # v    (node-major tiles)

            stateT_sb = persist.tile([P, EL], BF16)
            for q4 in range(4):
                nc.sync.dma_start(
                    stateT_sb[:, q4 * 1024 : (q4 + 1) * 1024],
                    stateT_l[:, q4 * 1024 : (q4 + 1) * 1024],
                )
            nc.sync.dma_start(featT[:], featT_l[:])

            # ------- phase 0 (memory MLP) interleaved with phase 1 -------
            # phase-1 accumulators: v = (-s).T @ mT  in [D, N] layout.
            accs = [
                ps_acc.tile([P, 512], F32, tag=f"acc{q}", name=f"p1acc{q}")
                for q in range(4)
            ]
            for g in range(NCH // 2):
                pj = (2 * g, 2 * g + 1)
                h1s = {}
                for j in pj:
                    h1 = ps_mm.tile([P, 512], F32, tag="mm", name=f"h1_{j}")
                    nc.tensor.matmul(
                        h1[:], w1mT[:], stateT_sb[:, j * 512 : (j + 1) * 512],
                        start=True, stop=True,
                    )
                    h1s[j] = h1
                ex1s = {}
                for j in pj:
                    ex1 = tmp.tile([P, 512], F32, tag="ex", name=f"ex1_{j}")
                    nc.scalar.activation(ex1[:], h1s[j][:], AF.Exp,
                                         scale=-1.0, bias=nb1m_sb[:])
                    ex1s[j] = ex1
                u1s = {}
                for j in pj:
                    u1 = tmp.tile([P, 512], BF16, tag="u1", name=f"u1_{j}")
                    nc.scalar.activation(u1[:], ex1s[j][:], AF.Ln, bias=1.0)
                    u1s[j] = u1
                z2s = {}
                for j in pj:
                    z2 = ps_mm.tile([P, 512], F32, tag="mm", name=f"z2_{j}")
                    nc.tensor.matmul(z2[:], w2mnT[:], u1s[j][:],
                                     start=True, stop=True)
                    z2s[j] = z2
                ex2s = {}
                for j in pj:
                    ex2 = tmp.tile([P, 512], F32, tag="ex2", name=f"ex2_{j}")
                    nc.scalar.activation(ex2[:], z2s[j][:], AF.Exp,
                                         scale=-1.0)
                    ex2s[j] = ex2
                for j in pj:
                    nc.scalar.activation(
                        u2T[:, j * 512 : (j + 1) * 512], ex2s[j][:],
                        AF.Ln, bias=1.0,
                    )
                for j in pj:
                    tp2 = ps_tp.tile([P, 512], BF16, tag="tp",
                                     name=f"tp2_{j}")
                    for k in range(4):
                        c0 = (j * 4 + k) * P
                        nc.tensor.transpose(
                            tp2[:, k * P : (k + 1) * P],
                            u2T[:, c0 : c0 + P],
                            idn_bf[:],
                        )
                    nc.vector.tensor_copy(
                        u2e[:, j * 4 : (j + 1) * 4, :].rearrange(
                            "p a d -> p (a d)"
                        ),
                        tp2[:],
                    )
                    for t_i in range(4 * j, 4 * j + 4):
                        mt = mtp.tile([P, N], FP8, tag="mt",
                                      name=f"mt_{t_i}")
                        nc.sync.dma_start(
                            mt[:], mT_l[t_i * P : (t_i + 1) * P, :]
                        )
                        for q in range(4):
                            nc.tensor.matmul(
                                accs[q][:],
                                u2e[:, t_i, :],
                                mt[:, q * 512 : (q + 1) * 512],
                                start=(t_i == 0),
                                stop=(t_i == NT - 1),
                            )

            # ---- prefetch full mask into SBUF (behind mT in queue order);
            # streams during the tail of phase 1 and the collective ----
            msk = []
            for i in range(NNCH):
                mk = maskp.tile([P, EL], FP8, name=f"mask_{i}")
                nc.sync.dma_start(mk[:], mask_l[i * P : (i + 1) * P, :])
                msk.append(mk)

            # ---------------- AllReduce (single, bf16) ----------------
            vsb = persist.tile([P, N], BF16)
            for q in range(4):
                nc.vector.tensor_copy(
                    vsb[:, q * 512 : (q + 1) * 512], accs[q][:]
                )
            cc_in = dram.tile([P, N], BF16)
            cc_out = dram.tile([P, N], BF16, addr_space="Shared")
            for hv in range(4):
                nc.gpsimd.dma_start(
                    cc_in[:, hv * 512 : (hv + 1) * 512],
                    vsb[:, hv * 512 : (hv + 1) * 512],
                )
            nc.gpsimd.collective_compute(
                "AllReduce",
                mybir.AluOpType.add,
                ins=[cc_in.opt()],
                outs=[cc_out.opt()],
                replica_groups=[list(range(N_CORES))],
            )
            vfull = persist.tile([P, N], BF16)
            for hv in range(4):
                nc.gpsimd.dma_start(
                    vfull[:, hv * 512 : (hv + 1) * 512],
                    cc_out[:, hv * 512 : (hv + 1) * 512],
                )

            for g in range(4):
                tp3 = ps_tp.tile([P, 512], BF16, tag="tp",
                                 name=f"tp3_{g}")
                for k in range(4):
                    i = g * 4 + k
                    nc.tensor.transpose(
                        tp3[:, k * P : (k + 1) * P],
                        vfull[:, i * P : (i + 1) * P],
                        idn_bf[:],
                    )
                nc.vector.tensor_copy(
                    vT[:, g * 4 : (g + 1) * 4, :].rearrange("p a d -> p (a d)"),
                    tp3[:],
                )

            # ---------------- phase 2: edge agg + concat MLP ----------------
            out_v = out_l.rearrange("(c k p) d -> c p k d", k=4, p=P)

            def p2_mlp_pair(jacc):
                """Concat-MLP for two 512-edge chunks given their (negated)
                aggregate accs.  All softplus sites are wide-range safe."""
                w3s, z1as, ts, as_, u3s = {}, {}, {}, {}, {}
                for j, acc in jacc:
                    w3 = tmp.tile([P, 512], BF16, tag="w3", name=f"w3_{j}")
                    nc.vector.tensor_sub(
                        w3[:], acc[:], u2T[:, j * 512 : (j + 1) * 512]
                    )
                    w3s[j] = w3
                for j, acc in jacc:
                    z1a = ps_mm.tile([P, 512], F32, tag="mm", name=f"z1a_{j}")
                    nc.tensor.matmul(z1a[:], w1anT[:], w3s[j][:],
                                     start=True, stop=False)
                    nc.tensor.matmul(
                        z1a[:], wa2T[:], featT[:, j * 512 : (j + 1) * 512],
                        start=False, stop=True,
                    )
                    z1as[j] = z1a
                for j, acc in jacc:
                    t = tmp.tile([P, 512], F32, tag="t", name=f"t_{j}")
                    nc.vector.tensor_scalar(
                        t[:], z1as[j][:], -1.0, b1a_sb[:], ALU.mult,
                        ALU.subtract,
                    )
                    a = tmp.tile([P, 512], F32, tag="a", name=f"a_{j}")
                    nc.vector.tensor_scalar(
                        a[:].bitcast(mybir.dt.uint32),
                        t[:].bitcast(mybir.dt.uint32),
                        0x7FFFFFFF, None, ALU.bitwise_and,
                    )
                    ts[j], as_[j] = t, a
                exs = {}
                for j, acc in jacc:
                    ex = tmp.tile([P, 512], F32, tag="ex", name=f"exa_{j}")
                    nc.scalar.activation(ex[:], as_[j][:], AF.Exp,
                                         scale=-1.0)
                    exs[j] = ex
                sps = {}
                for j, acc in jacc:
                    sp = tmp.tile([P, 512], F32, tag="sp", name=f"sp_{j}")
                    nc.scalar.activation(sp[:], exs[j][:], AF.Ln, bias=1.0)
                    sps[j] = sp
                for j, acc in jacc:
                    u3 = tmp.tile([P, 512], BF16, tag="u3", name=f"u3_{j}")
                    nc.vector.scalar_tensor_tensor(
                        u3[:], ts[j][:], 0.0, sps[j][:], ALU.max, ALU.add
                    )
                    u3s[j] = u3
                pos, a2s, l2s = {}, {}, {}
                for j, acc in jacc:
                    po = ps_mm.tile([P, 512], F32, tag="mm", name=f"po_{j}")
                    for k in range(4):
                        nc.tensor.matmul(
                            po[:, k * P : (k + 1) * P],
                            u3s[j][:, k * P : (k + 1) * P],
                            w2anT[:],
                            start=True,
                            stop=True,
                        )
                    pos[j] = po
                    a2 = tmp.tile([P, 512], F32, tag="a", name=f"a2_{j}")
                    nc.vector.tensor_scalar(
                        a2[:].bitcast(mybir.dt.uint32),
                        po[:].bitcast(mybir.dt.uint32),
                        0x7FFFFFFF, None, ALU.bitwise_and,
                    )
                    a2s[j] = a2
                e2s = {}
                for j, acc in jacc:
                    e2 = tmp.tile([P, 512], F32, tag="ex", name=f"e2_{j}")
                    nc.scalar.activation(e2[:], a2s[j][:], AF.Exp,
                                         scale=-1.0)
                    e2s[j] = e2
                for j, acc in jacc:
                    l2 = tmp.tile([P, 512], F32, tag="sp", name=f"l2_{j}")
                    nc.scalar.activation(l2[:], e2s[j][:], AF.Ln, bias=1.0)
                    l2s[j] = l2
                for j, acc in jacc:
                    ob = outp.tile([P, 512], F32, tag="ob", name=f"ob_{j}")
                    nc.vector.scalar_tensor_tensor(
                        ob[:], pos[j][:], 0.0, l2s[j][:], ALU.min,
                        ALU.subtract,
                    )
                    nc.gpsimd.dma_start(
                        out_v[j], ob.rearrange("p (k d) -> p k d", k=4)
                    )

            # edge-halves of 2048; accumulate over all 16 node chunks from
            # the SBUF-resident mask, then run the MLP on 4 chunks of 512.
            for eh in range(2):
                js = tuple(4 * eh + q for q in range(4))
                acc_w = {
                    j: ps_acc.tile([P, 512], F32, tag=f"acc{j % 4}",
                                   name=f"p2acc_{j}")
                    for j in js
                }
                for nch in range(NNCH):
                    for ji, j in enumerate(js):
                        nc.tensor.matmul(
                            acc_w[j][:],
                            vT[:, nch, :],
                            msk[nch][
                                :, eh * 2048 + ji * 512
                                : eh * 2048 + (ji + 1) * 512
                            ],
                            start=(nch == 0),
                            stop=(nch == NNCH - 1),
                        )
                p2_mlp_pair([(j, acc_w[j]) for j in js[:2]])
                p2_mlp_pair([(j, acc_w[j]) for j in js[2:]])
    nc.compile()
    return nc


def kernel(**inputs: np.ndarray) -> np.ndarray:
    from concourse.bass_utils import run_bass_kernel_spmd

    if "nc" not in _CACHE:
        _CACHE["nc"] = _build()
    nc = _CACHE["nc"]

    state = np.ascontiguousarray(inputs["state"], dtype=np.float32)
    feature = np.ascontiguousarray(inputs["feature"], dtype=np.float32)
    mask = np.ascontiguousarray(inputs["mask"], dtype=np.float32)
    mask_transpose = np.ascontiguousarray(
        inputs["mask_transpose"], dtype=np.float32
    )
    idn_np = np.eye(P, dtype=np.float32)

    common = {
        "w1m": np.ascontiguousarray(inputs["W1_m"], dtype=np.float32),
        "b1m": np.ascontiguousarray(inputs["b1_m"], dtype=np.float32),
        "w2m": np.ascontiguousarray(inputs["W2_m"], dtype=np.float32),
        "w1a": np.ascontiguousarray(inputs["W1_a"], dtype=np.float32),
        "b1a": np.ascontiguousarray(inputs["b1_a"], dtype=np.float32),
        "w2a": np.ascontiguousarray(inputs["W2_a"], dtype=np.float32),
        "idn": idn_np,
    }
    in_maps = []
    for c in range(N_CORES):
        sl = slice(c * EL, (c + 1) * EL)
        in_maps.append(
            {
                "stateT_l": np.ascontiguousarray(state[sl].T).astype(
                    ml_dtypes.bfloat16
                ),
                "featT_l": np.ascontiguousarray(feature[sl].T).astype(
                    ml_dtypes.bfloat16
                ),
                "mT_l": mask_transpose[sl].astype(ml_dtypes.float8_e4m3fn),
                "mask_l": np.ascontiguousarray(mask[:, sl]).astype(
                    ml_dtypes.float8_e4m3fn
                ),
                **common,
            }
        )
    _CACHE["in_maps"] = in_maps

    res = run_bass_kernel_spmd(nc, in_maps, core_ids=list(range(N_CORES)))
    out = np.concatenate(
        [res.results[c]["out_l"] for c in range(N_CORES)], axis=0
    )
    return out
